# revision 35
# baseline (speedup 1.0000x reference)
"""DeepSet election model on 8 Trainium2 NeuronCores.

Strategy (exploits the *sorted* index): rows are sharded by SEGMENT
OWNERSHIP - core k owns elections [512k, 512(k+1)), so no collective is
needed.  Rows are laid out window-contiguous (window = 32 consecutive
segments) with per-window padding to a common w_rows.

Per core pipeline (fp8 e4m3 data path, f32 PSUM accumulation):
  1. L1:   h1T[128emb, rows] = W1.T @ x    as fp8 DoubleRow matmuls
           (x host-packed [16, 2, rows]: K=32 contraction as 16 partitions
           x 2 k-tiles, 0.5 cyc/row)
  2. relu1 evac PSUM->SBUF fp8 (+lb1)      [ACT/DVE/POOL greedy balance]
  3. L2:   h2[rows, emb] = h1T_chunk.T @ w2 (fp8 moving, 1 cyc/row)
  4. relu2 evac PSUM->SBUF fp8             [ACT/DVE/POOL greedy balance]
  5. segment sums: fp8 DoubleRow over chunk PAIRS (K=256 rows/instr,
     0.5 cyc/row, N=32 segs): aggT windows [128emb, 32seg] in PSUM
  6. deferred local layer 3 + global MLP + log_softmax on [512, 128].

DMA: x and one-hot S_row are shipped fp8, batched G quads per transfer
(HWDGE is a serial ~625ns/DMA device - fewer, bigger DMAs).
"""

import math
from contextlib import ExitStack

import numpy as np
import ml_dtypes

import concourse.bass as bass
import concourse.bacc as bacc
import concourse.mybir as mybir
import concourse.tile as tile
from concourse import bass_utils

BF16 = mybir.dt.bfloat16
F32 = mybir.dt.float32
F8 = mybir.dt.float8e4
AF = mybir.ActivationFunctionType
ALU = mybir.AluOpType
DR = mybir.MatmulPerfMode.DoubleRow

N_VOTERS = 1048576
NUM_ELECTIONS = 4096
C = 32     # candidates
E = 128    # embedding width
N_CORES = 8
SEGS_PER_CORE = NUM_ELECTIONS // N_CORES   # 512
W_SEGS = 32                                # segments per PSUM window
N_WINDOWS = SEGS_PER_CORE // W_SEGS        # 16

_nb16 = lambda a: np.ascontiguousarray(a).astype(ml_dtypes.bfloat16)
_np8 = mybir.dt.np(F8)
_nf8 = lambda a: np.ascontiguousarray(a).astype(_np8)


def _pick_group(n_quads: int) -> int:
    for g in (8, 7, 6, 5, 4, 3, 2, 1):
        if n_quads % g == 0:
            return g
    return 1


def _build_program(w_rows: int):
    """Build + compile the SPMD Bass program. w_rows = padded rows per
    (core, window); multiple of 256. Identical structure on every core."""
    assert w_rows % 256 == 0
    R = N_WINDOWS * w_rows                  # rows per core
    n_quads = R // 2048
    assert n_quads * 2048 == R
    ppw = w_rows // 256                     # chunk-pairs per window
    # DMA group plan: two small leading groups so the pipeline starts fast,
    # then steady groups of G quads
    G = _pick_group(n_quads)                # steady quads per DMA group
    groups = [1, 1] if n_quads > 2 else []
    rest = n_quads - sum(groups)
    while rest:
        g = min(G, rest)
        groups.append(g)
        rest -= g

    nc = bacc.Bacc(
        "TRN2",
        target_bir_lowering=False,
        debug=False,
        enable_asserts=True,
        num_devices=N_CORES,
    )

    dt_in = lambda n, sh, dt: nc.dram_tensor(n, sh, dt, kind="ExternalInput").ap()
    # x packed for DoubleRow: per quad [16, 2, 2048] (c = i*16 + k), all
    # quads concatenated along the free dim
    xt4 = dt_in("xt4", [16, n_quads * 4096], F8)
    # one-hot rows vs window-relative seg id, chunk-major: per quad
    # [128, 16 chunks * W_SEGS]
    srowd = dt_in("srow", [128, n_quads * 16 * W_SEGS], F8)
    w1pd = dt_in("w1p", [16, 256], F8)      # w1p[k, i*128+e] = W1[i*16+k, e]
    lw2 = dt_in("lw2", [E, E], F8)
    lw3 = dt_in("lw3", [E, E], BF16)
    gw1 = dt_in("gw1", [E, E], BF16)
    gw2 = dt_in("gw2", [E, E], BF16)
    gw3 = dt_in("gw3", [E, C], BF16)
    identf = dt_in("identf", [128, 128], F32)
    lb1d = dt_in("lb1", [E, 1], F32)
    gb1d = dt_in("gb1", [E, 1], F32)
    gb2d = dt_in("gb2", [E, 1], F32)
    gb3d = dt_in("gb3", [C, 1], F32)
    out_ap = nc.dram_tensor("out", [SEGS_PER_CORE, C], F32, kind="ExternalOutput").ap()

    with tile.TileContext(nc) as tc:
        with ExitStack() as octx:
            cpool = octx.enter_context(tc.tile_pool(name="const", bufs=1))
            aggps = octx.enter_context(tc.tile_pool(name="aggps", bufs=1, space="PSUM"))
            tailp = octx.enter_context(tc.tile_pool(name="tail", bufs=2))

            def cload(ap, shape, dtype, tag):
                t = cpool.tile(shape, dtype, tag=tag)
                nc.sync.dma_start(t[:], ap[:])
                return t

            # critical-path constants only; tail-only constants are DMA'd
            # after the main loop
            w1p = cload(w1pd, [16, 256], F8, "w1p")
            w2 = cload(lw2, [E, E], F8, "w2")
            lb1 = cload(lb1d, [E, 1], F32, "lb1")

            # aggT: [128 emb, 512 seg] bf16 segment sums (pre-transposed for
            # the tail)
            aggT = cpool.tile([128, 4 * E], BF16, tag="aggT")

            # greedy 3-lane balancing, readiness-aware: track a modeled PE
            # clock (when each PSUM source is produced) and assign each
            # evacuation to the engine with the earliest achievable finish
            lane_t = {"act": 0.0, "dve": 0.0}
            pe_clock = [0.0]

            def evac(dst, src, rows, relu, bias=None, engine=None):
                costs = {
                    "act": rows * 0.8333 + 143.0,
                    "dve": rows * 1.0417 + 125.0,
                }
                e = engine or min(lane_t, key=lambda k: lane_t[k] + costs[k])
                lane_t[e] += costs[e]
                if e == "act":
                    if relu:
                        nc.scalar.activation(dst, src, AF.Relu,
                                             bias=bias[:] if bias is not None else 0.0)
                    else:
                        nc.scalar.copy(dst, src)
                elif e == "dve":
                    if relu and bias is not None:
                        nc.vector.tensor_scalar(dst, src, bias[:], 0.0,
                                                ALU.add, ALU.max)
                    elif relu:
                        nc.vector.tensor_scalar_max(dst, src, 0.0)
                    else:
                        nc.vector.tensor_copy(dst, src)
                else:
                    if relu and bias is not None:
                        nc.gpsimd.tensor_scalar(dst, src, bias[:], 0.0,
                                                ALU.add, ALU.max)
                    elif relu:
                        nc.gpsimd.tensor_scalar_max(dst, src, 0.0)
                    else:
                        nc.gpsimd.tensor_copy(dst, src)

            # ================= main per-row loop =================
            # Software-pipelined: at step q we emit L1 for quad q, L2 for
            # quad q-1, and segment-sums for quad q-2, so no PE instruction
            # ever waits on a lane evacuation issued in the same step.
            with ExitStack() as ictx:
                xtp = ictx.enter_context(tc.tile_pool(name="xt", bufs=2))
                srp = ictx.enter_context(tc.tile_pool(name="sr", bufs=3))
                l1ps = ictx.enter_context(tc.tile_pool(name="l1ps", bufs=2, space="PSUM"))
                h1p = ictx.enter_context(tc.tile_pool(name="h1", bufs=6))
                l2ps = ictx.enter_context(tc.tile_pool(name="l2ps", bufs=3, space="PSUM"))
                h2p = ictx.enter_context(tc.tile_pool(name="h2", bufs=10))

                w1v = w1p[:].rearrange("k (i e) -> k i e", i=2)
                # one persistent PSUM bank holds all 16 aggregation windows
                agg_tile = aggps.tile([E, N_WINDOWS * W_SEGS], F32, tag="agg")
                h1_of = {}     # q -> [h1 half tiles]
                h2_of = {}     # q -> [h2 tiles]
                srv_of = {}    # q -> (srv group view, ql)

                def do_L1(q, xtv, ql):
                    h1s = []
                    for h in range(2):
                        l1 = l1ps.tile([128, 1024], F32, tag="l1")
                        for m in range(4):
                            r0 = 1024 * h + 256 * m
                            nc.tensor.matmul(
                                l1[:, 256 * m:256 * m + 256],
                                w1v,
                                xtv[:, 2 * ql:2 * ql + 2, r0:r0 + 256],
                                start=True, stop=True, perf_mode=DR,
                            )
                            pe_clock[0] += 53.3
                        h1 = h1p.tile([128, 1024], F8, tag="h1")
                        # relu1 (1024-row ops) always on ACT: its per-row
                        # cost there is lowest and it keeps DVE for the
                        # smaller relu2 ops
                        evac(h1[:], l1[:], 1024, relu=True, bias=lb1,
                             engine="act")
                        h1s.append(h1)
                    h1_of[q] = h1s

                def do_L2(q, i):
                    h1 = h1_of[q][i // 2]
                    hoff = 512 * (i % 2)
                    l2 = l2ps.tile([128, 512], F32, tag="l2")
                    for c4 in range(4):
                        nc.tensor.matmul(
                            l2[:, 128 * c4:128 * c4 + 128],
                            h1[:, hoff + 128 * c4:hoff + 128 * c4 + 128],
                            w2[:],
                            start=True, stop=True,
                        )
                        pe_clock[0] += 53.3
                    h2 = h2p.tile([128, 512], F8, tag="h2")
                    evac(h2[:], l2[:], 512, relu=True)
                    h2_of.setdefault(q, []).append(h2)

                def do_seg(q, i):
                    srv, ql = srv_of[q]
                    h2v = h2_of[q][i][:].rearrange("p (i m) -> p i m", i=4)
                    for p2 in range(2):
                        cpid = (q * 4 + i) * 2 + p2
                        w = cpid // ppw
                        first = (cpid % ppw == 0)
                        last = (cpid % ppw == ppw - 1)
                        # aggT_w += h2_pair.T @ S_row_pair (fp8 DoubleRow:
                        # K = 256 rows in one instruction).  All 16 windows
                        # live side by side in one persistent PSUM bank;
                        # start only zeroes the bytes this matmul writes.
                        nc.tensor.matmul(
                            agg_tile[:, W_SEGS * w:W_SEGS * (w + 1)],
                            h2v[:, 2 * p2:2 * p2 + 2, :],
                            srv[:, 16 * ql + 4 * i + 2 * p2:
                                16 * ql + 4 * i + 2 * p2 + 2, :],
                            start=first, stop=last, perf_mode=DR,
                        )
                        pe_clock[0] += 6.7
                        if last and w == N_WINDOWS - 1:
                            evac(aggT[:], agg_tile[:], 512, relu=False,
                                 engine="dve")

                SRQ = 16 * W_SEGS
                group_start = {}
                s = 0
                for g in groups:
                    group_start[s] = g
                    s += g

                for q in range(n_quads + 2):
                    if q < n_quads:
                        if q in group_start:
                            g = group_start[q]
                            q0 = q
                            xt = xtp.tile([16, G * 4096], F8, tag="xt")
                            nc.sync.dma_start(
                                xt[:, :g * 4096],
                                xt4[:, q * 4096:(q + g) * 4096])
                            sr = srp.tile([128, G * SRQ], F8, tag="sr")
                            nc.sync.dma_start(
                                sr[:, :g * SRQ],
                                srowd[:, q * SRQ:(q + g) * SRQ])
                            xtv = xt[:].rearrange("k (q r) -> k q r", r=2048)
                            srv = sr[:].rearrange("p (c j) -> p c j", j=W_SEGS)
                        srv_of[q] = (srv, q - q0)
                        do_L1(q, xtv, q - q0)
                    for i in range(4):
                        if 1 <= q < n_quads + 1:
                            do_L2(q - 1, i)
                        if q >= 2:
                            do_seg(q - 2, i)
                    h1_of.pop(q - 2, None)
                    h2_of.pop(q - 3, None)
                    srv_of.pop(q - 3, None)

            # tail-only constants (deferred DMAs)
            w3 = cload(lw3, [E, E], BF16, "w3")
            g1w = cload(gw1, [E, E], BF16, "g1w")
            g2w = cload(gw2, [E, E], BF16, "g2w")
            g3w = cload(gw3, [E, C], BF16, "g3w")
            idf = cload(identf, [128, 128], F32, "idf")
            gb1 = cload(gb1d, [E, 1], F32, "gb1")
            gb2 = cload(gb2d, [E, 1], F32, "gb2")
            gb3 = cload(gb3d, [C, 1], F32, "gb3")

            # ---------- tail: deferred layer-3 + global MLP ----------
            with ExitStack() as tctx:
                tailps = tctx.enter_context(
                    tc.tile_pool(name="tailps", bufs=2, space="PSUM"))

                ec_flip = 0

                def evac_copy(dst, src):
                    nonlocal ec_flip
                    ec_flip += 1
                    if ec_flip % 2:
                        nc.scalar.copy(dst, src)
                    else:
                        nc.vector.tensor_copy(dst, src)

                # half-split column-pipelined global MLP: the 512 segment
                # columns are independent, so run two 256-column chains and
                # interleave layers for latency
                def emit_half(rhs_tile, w_tile, func, bias, o, hh,
                              out_cols=E):
                    ps = tailps.tile([out_cols, 256], F32, tag="lps")
                    nc.tensor.matmul(ps[:], w_tile[:],
                                     rhs_tile[:, 256 * hh:256 * hh + 256],
                                     start=True, stop=True)
                    dst = o[:, 256 * hh:256 * hh + 256]
                    if func is None:
                        evac_copy(dst, ps[:])
                    elif func == AF.Relu:
                        if bias is not None:
                            nc.vector.tensor_scalar(dst, ps[:], bias[:], 0.0,
                                                    ALU.add, ALU.max)
                        else:
                            nc.vector.tensor_scalar_max(dst, ps[:], 0.0)
                    elif func == AF.Identity and bias is not None:
                        nc.vector.tensor_scalar_add(dst, ps[:], bias[:])
                    else:
                        nc.scalar.activation(
                            dst, ps[:], func,
                            bias=bias[:] if bias is not None else 0.0)

                a3T = tailp.tile([E, 512], BF16, tag="a3T")
                g1T = tailp.tile([E, 512], BF16, tag="g1T")
                g2T = tailp.tile([E, 512], BF16, tag="g2T")
                scT = tailp.tile([C, 512], F32, tag="scT")
                chain = [
                    (aggT, w3, None, None, a3T, E),
                    (a3T, g1w, AF.Relu, gb1, g1T, E),
                    (g1T, g2w, AF.Relu, gb2, g2T, E),
                    (g2T, g3w, AF.Identity, gb3, scT, C),
                ]
                for step in range(5):
                    for li, (rhs, wt, fn, bi, o, ocols) in enumerate(chain):
                        hh = step - li
                        if hh in (0, 1):
                            emit_half(rhs, wt, fn, bi, o, hh, out_cols=ocols)

                # log-softmax, phase-batched so ACT loads the exp table once
                # and the ln table once
                outsb = tailp.tile([128, 4 * C], F32, tag="outsb")
                xs = tailp.tile([128, 4 * C], F32, tag="xs")
                exs = tailp.tile([128, 4 * C], F32, tag="exs")
                negmax = tailp.tile([128, 4], F32, tag="negmax")
                ssum = tailp.tile([128, 4], F32, tag="ssum")
                lse = tailp.tile([128, 4], F32, tag="lse")
                shift = tailp.tile([128, 4], F32, tag="shift")
                mx = tailp.tile([128, 4], F32, tag="mx")
                for t in range(4):
                    sp = tailps.tile([128, C], F32, tag="sp")
                    nc.tensor.transpose(sp[:], scT[:, 128 * t:128 * t + 128],
                                        idf[:C, :C])
                    nc.vector.tensor_copy(xs[:, C * t:C * (t + 1)], sp[:])
                    nc.vector.tensor_reduce(mx[:, t:t + 1],
                                            xs[:, C * t:C * (t + 1)],
                                            mybir.AxisListType.X, ALU.max)
                for t in range(4):
                    nc.vector.tensor_scalar_mul(negmax[:, t:t + 1],
                                                mx[:, t:t + 1], -1.0)
                    nc.scalar.activation(exs[:, C * t:C * (t + 1)],
                                         xs[:, C * t:C * (t + 1)], AF.Exp,
                                         bias=negmax[:, t:t + 1])
                    nc.vector.reduce_sum(ssum[:, t:t + 1],
                                         exs[:, C * t:C * (t + 1)],
                                         axis=mybir.AxisListType.X)
                nc.scalar.activation(lse[:], ssum[:], AF.Ln)
                nc.vector.tensor_tensor(shift[:], negmax[:], lse[:],
                                        op=ALU.subtract)
                for t in range(4):
                    nc.vector.tensor_scalar_add(outsb[:, C * t:C * (t + 1)],
                                                xs[:, C * t:C * (t + 1)],
                                                shift[:, t:t + 1])

                outv = out_ap.rearrange("(t p) c -> p t c", p=128)
                nc.sync.dma_start(
                    outv, outsb[:].rearrange("p (t c) -> p t c", c=C))

    nc.compile()
    return nc, G, R


def _prep_core(x, index_local, core, w_rows, R):
    """Per-core xt4 + srow tensors (fp8)."""
    segs0 = core * SEGS_PER_CORE
    seg_of_row = index_local - segs0

    # destination row: window-contiguous with per-window padding to w_rows
    win_of_row = seg_of_row // W_SEGS
    win_cnt = np.bincount(win_of_row, minlength=N_WINDOWS)
    win_orig_start = np.concatenate(([0], np.cumsum(win_cnt)[:-1]))
    dest = win_of_row * w_rows + (np.arange(len(index_local))
                                  - win_orig_start[win_of_row])
    xpad = np.zeros((R, C), dtype=np.float32)
    xpad[dest] = x

    n_quads = R // 2048
    # x packed for DoubleRow: per quad [16, 2, 2048] (c = i*16 + k),
    # flattened to [16, n_quads*4096]
    xq = xpad.reshape(n_quads, 2048, 2, 16).transpose(0, 3, 2, 1)
    xt4 = xq.transpose(1, 0, 2, 3).reshape(16, n_quads * 4096)

    # per-row one-hot vs window-relative segment id (pad rows get all-zero)
    d = np.full(R, -(10 ** 6), dtype=np.int64)
    d[dest] = seg_of_row - win_of_row * W_SEGS
    srow = (d[:, None] == np.arange(W_SEGS)[None, :])      # [R, 32]
    n_chunks = R // 128
    srow = srow.reshape(n_chunks, 128, W_SEGS).transpose(1, 0, 2)
    srow = srow.reshape(128, n_chunks * W_SEGS)
    return _nf8(xt4), _nf8(srow.astype(np.float32))


def kernel(**inputs) -> np.ndarray:
    x = np.asarray(inputs["x"], dtype=np.float32)
    index = np.asarray(inputs["index"]).astype(np.int64)
    ws = {k: np.asarray(inputs[k], dtype=np.float32)
          for k in ("lW1", "lb1", "lW2", "lb2", "lW3", "lb3",
                    "gW1", "gb1", "gW2", "gb2", "gW3", "gb3")}

    # lb2 enters per-row on the free axis, lb3 would need per-segment counts;
    # both are zero for this model.
    assert not ws["lb2"].any() and not ws["lb3"].any(), \
        "nonzero lb2/lb3 not supported by this kernel"

    if not np.all(index[:-1] <= index[1:]):
        order = np.argsort(index, kind="stable")
        index = index[order]
        x = x[order]

    counts = np.bincount(index, minlength=NUM_ELECTIONS)
    ptr = np.concatenate(([0], np.cumsum(counts)))

    # rows per (core, window), padded to the global max (256-aligned)
    win_rows = counts.reshape(N_CORES * N_WINDOWS, W_SEGS).sum(axis=1)
    w_rows = int(-(-win_rows.max() // 256) * 256)

    nc, G, R = _build_program(w_rows)

    # w1p[k, i*128+e] = W1[i*16+k, e]
    w1p = ws["lW1"].reshape(2, 16, E).transpose(1, 0, 2).reshape(16, 256)

    common = {
        "w1p": _nf8(w1p),
        "lw2": _nf8(ws["lW2"]),
        "lw3": _nb16(ws["lW3"]),
        "gw1": _nb16(ws["gW1"]),
        "gw2": _nb16(ws["gW2"]),
        "gw3": _nb16(ws["gW3"]),
        "identf": np.eye(128, dtype=np.float32),
        "lb1": ws["lb1"].reshape(E, 1).astype(np.float32),
        "gb1": ws["gb1"].reshape(E, 1).astype(np.float32),
        "gb2": ws["gb2"].reshape(E, 1).astype(np.float32),
        "gb3": ws["gb3"].reshape(C, 1).astype(np.float32),
    }

    in_maps = []
    for k in range(N_CORES):
        lo, hi = ptr[k * SEGS_PER_CORE], ptr[(k + 1) * SEGS_PER_CORE]
        xt4, srow = _prep_core(x[lo:hi], index[lo:hi], k, w_rows, R)
        in_maps.append({"xt4": xt4, "srow": srow, **common})

    res = bass_utils.run_bass_kernel_spmd(nc, in_maps, core_ids=list(range(N_CORES)))
    global LAST_RESULTS, LAST_NC, LAST_IN_MAPS
    LAST_RESULTS, LAST_NC, LAST_IN_MAPS = res, nc, in_maps
    out = np.concatenate([res.results[k]["out"] for k in range(N_CORES)], axis=0)
    return out.astype(np.float32)


LAST_RESULTS = None
LAST_NC = None
LAST_IN_MAPS = None


if __name__ == "__main__":
    rng = np.random.default_rng(0)
    idx = np.sort(rng.integers(0, NUM_ELECTIONS, size=N_VOTERS)).astype(np.int64)
    demo = {
        "x": rng.standard_normal((N_VOTERS, C), dtype=np.float32),
        "index": idx,
    }
    for n, sh in (("lW1", (C, E)), ("lW2", (E, E)), ("lW3", (E, E)),
                  ("gW1", (E, E)), ("gW2", (E, E)), ("gW3", (E, C))):
        demo[n] = (rng.standard_normal(sh, dtype=np.float32) * 0.05)
    for n, sh in (("lb1", E), ("lb2", E), ("lb3", E),
                  ("gb1", E), ("gb2", E), ("gb3", C)):
        demo[n] = np.zeros(sh, np.float32)
    out = kernel(**demo)
    print(out.shape, out.dtype, np.isfinite(out).all())


# revision 36
# speedup vs baseline: 1.0072x; 1.0072x over previous
"""DeepSet election model on 8 Trainium2 NeuronCores.

Strategy (exploits the *sorted* index): rows are sharded by SEGMENT
OWNERSHIP - core k owns elections [512k, 512(k+1)), so no collective is
needed.  Rows are laid out window-contiguous (window = 32 consecutive
segments) with per-window padding to a common w_rows.

Per core pipeline (fp8 e4m3 data path, f32 PSUM accumulation):
  1. L1:   h1T[128emb, rows] = W1.T @ x    as fp8 DoubleRow matmuls
           (x host-packed [16, 2, rows]: K=32 contraction as 16 partitions
           x 2 k-tiles, 0.5 cyc/row)
  2. relu1 evac PSUM->SBUF fp8 (+lb1)      [ACT/DVE/POOL greedy balance]
  3. L2:   h2[rows, emb] = h1T_chunk.T @ w2 (fp8 moving, 1 cyc/row)
  4. relu2 evac PSUM->SBUF fp8             [ACT/DVE/POOL greedy balance]
  5. segment sums: fp8 DoubleRow over chunk PAIRS (K=256 rows/instr,
     0.5 cyc/row, N=32 segs): aggT windows [128emb, 32seg] in PSUM
  6. deferred local layer 3 + global MLP + log_softmax on [512, 128].

DMA: x and one-hot S_row are shipped fp8, batched G quads per transfer
(HWDGE is a serial ~625ns/DMA device - fewer, bigger DMAs).
"""

import math
from contextlib import ExitStack

import numpy as np
import ml_dtypes

import concourse.bass as bass
import concourse.bacc as bacc
import concourse.mybir as mybir
import concourse.tile as tile
from concourse import bass_utils

BF16 = mybir.dt.bfloat16
F32 = mybir.dt.float32
F8 = mybir.dt.float8e4
AF = mybir.ActivationFunctionType
ALU = mybir.AluOpType
DR = mybir.MatmulPerfMode.DoubleRow

N_VOTERS = 1048576
NUM_ELECTIONS = 4096
C = 32     # candidates
E = 128    # embedding width
N_CORES = 8
SEGS_PER_CORE = NUM_ELECTIONS // N_CORES   # 512
W_SEGS = 32                                # segments per PSUM window
N_WINDOWS = SEGS_PER_CORE // W_SEGS        # 16

_nb16 = lambda a: np.ascontiguousarray(a).astype(ml_dtypes.bfloat16)
_np8 = mybir.dt.np(F8)
_nf8 = lambda a: np.ascontiguousarray(a).astype(_np8)


def _pick_group(n_quads: int) -> int:
    for g in (8, 7, 6, 5, 4, 3, 2, 1):
        if n_quads % g == 0:
            return g
    return 1


def _build_program(w_rows: int):
    """Build + compile the SPMD Bass program. w_rows = padded rows per
    (core, window); multiple of 256. Identical structure on every core."""
    assert w_rows % 256 == 0
    R = N_WINDOWS * w_rows                  # rows per core
    n_quads = R // 2048
    assert n_quads * 2048 == R
    ppw = w_rows // 256                     # chunk-pairs per window
    # DMA group plan: two small leading groups so the pipeline starts fast,
    # then steady groups of G quads
    G = _pick_group(n_quads)                # steady quads per DMA group
    groups = []
    rest = n_quads - sum(groups)
    while rest:
        g = min(G, rest)
        groups.append(g)
        rest -= g

    nc = bacc.Bacc(
        "TRN2",
        target_bir_lowering=False,
        debug=False,
        enable_asserts=True,
        num_devices=N_CORES,
    )

    dt_in = lambda n, sh, dt: nc.dram_tensor(n, sh, dt, kind="ExternalInput").ap()
    # x packed for DoubleRow: per quad [16, 2, 2048] (c = i*16 + k), all
    # quads concatenated along the free dim
    xt4 = dt_in("xt4", [16, n_quads * 4096], F8)
    # one-hot rows vs window-relative seg id, chunk-major: per quad
    # [128, 16 chunks * W_SEGS]
    srowd = dt_in("srow", [128, n_quads * 16 * W_SEGS], F8)
    w1pd = dt_in("w1p", [16, 256], F8)      # w1p[k, i*128+e] = W1[i*16+k, e]
    lw2 = dt_in("lw2", [E, E], F8)
    lw3 = dt_in("lw3", [E, E], BF16)
    gw1 = dt_in("gw1", [E, E], BF16)
    gw2 = dt_in("gw2", [E, E], BF16)
    gw3 = dt_in("gw3", [E, C], BF16)
    identf = dt_in("identf", [128, 128], F32)
    lb1d = dt_in("lb1", [E, 1], F32)
    gb1d = dt_in("gb1", [E, 1], F32)
    gb2d = dt_in("gb2", [E, 1], F32)
    gb3d = dt_in("gb3", [C, 1], F32)
    out_ap = nc.dram_tensor("out", [SEGS_PER_CORE, C], F32, kind="ExternalOutput").ap()

    with tile.TileContext(nc) as tc:
        with ExitStack() as octx:
            cpool = octx.enter_context(tc.tile_pool(name="const", bufs=1))
            aggps = octx.enter_context(tc.tile_pool(name="aggps", bufs=1, space="PSUM"))
            tailp = octx.enter_context(tc.tile_pool(name="tail", bufs=2))

            def cload(ap, shape, dtype, tag):
                t = cpool.tile(shape, dtype, tag=tag)
                nc.sync.dma_start(t[:], ap[:])
                return t

            # critical-path constants only; tail-only constants are DMA'd
            # after the main loop
            w1p = cload(w1pd, [16, 256], F8, "w1p")
            w2 = cload(lw2, [E, E], F8, "w2")
            lb1 = cload(lb1d, [E, 1], F32, "lb1")

            # aggT: [128 emb, 512 seg] bf16 segment sums (pre-transposed for
            # the tail)
            aggT = cpool.tile([128, 4 * E], BF16, tag="aggT")

            # greedy 3-lane balancing, readiness-aware: track a modeled PE
            # clock (when each PSUM source is produced) and assign each
            # evacuation to the engine with the earliest achievable finish
            lane_t = {"act": 0.0, "dve": 0.0}
            pe_clock = [0.0]

            def evac(dst, src, rows, relu, bias=None, engine=None):
                costs = {
                    "act": rows * 0.8333 + 143.0,
                    "dve": rows * 1.0417 + 125.0,
                }
                e = engine or min(lane_t, key=lambda k: lane_t[k] + costs[k])
                lane_t[e] += costs[e]
                if e == "act":
                    if relu:
                        nc.scalar.activation(dst, src, AF.Relu,
                                             bias=bias[:] if bias is not None else 0.0)
                    else:
                        nc.scalar.copy(dst, src)
                elif e == "dve":
                    if relu and bias is not None:
                        nc.vector.tensor_scalar(dst, src, bias[:], 0.0,
                                                ALU.add, ALU.max)
                    elif relu:
                        nc.vector.tensor_scalar_max(dst, src, 0.0)
                    else:
                        nc.vector.tensor_copy(dst, src)
                else:
                    if relu and bias is not None:
                        nc.gpsimd.tensor_scalar(dst, src, bias[:], 0.0,
                                                ALU.add, ALU.max)
                    elif relu:
                        nc.gpsimd.tensor_scalar_max(dst, src, 0.0)
                    else:
                        nc.gpsimd.tensor_copy(dst, src)

            # ================= main per-row loop =================
            # Software-pipelined: at step q we emit L1 for quad q, L2 for
            # quad q-1, and segment-sums for quad q-2, so no PE instruction
            # ever waits on a lane evacuation issued in the same step.
            with ExitStack() as ictx:
                xtp = ictx.enter_context(tc.tile_pool(name="xt", bufs=2))
                srp = ictx.enter_context(tc.tile_pool(name="sr", bufs=3))
                l1ps = ictx.enter_context(tc.tile_pool(name="l1ps", bufs=2, space="PSUM"))
                h1p = ictx.enter_context(tc.tile_pool(name="h1", bufs=6))
                l2ps = ictx.enter_context(tc.tile_pool(name="l2ps", bufs=3, space="PSUM"))
                h2p = ictx.enter_context(tc.tile_pool(name="h2", bufs=10))

                w1v = w1p[:].rearrange("k (i e) -> k i e", i=2)
                # one persistent PSUM bank holds all 16 aggregation windows
                agg_tile = aggps.tile([E, N_WINDOWS * W_SEGS], F32, tag="agg")
                h1_of = {}     # q -> [h1 half tiles]
                h2_of = {}     # q -> [h2 tiles]
                srv_of = {}    # q -> (srv group view, ql)

                def do_L1(q, xtv, ql):
                    h1s = []
                    for h in range(2):
                        l1 = l1ps.tile([128, 1024], F32, tag="l1")
                        for m in range(4):
                            r0 = 1024 * h + 256 * m
                            nc.tensor.matmul(
                                l1[:, 256 * m:256 * m + 256],
                                w1v,
                                xtv[:, 2 * ql:2 * ql + 2, r0:r0 + 256],
                                start=True, stop=True, perf_mode=DR,
                            )
                            pe_clock[0] += 53.3
                        h1 = h1p.tile([128, 1024], F8, tag="h1")
                        # relu1 (1024-row ops) always on ACT: its per-row
                        # cost there is lowest and it keeps DVE for the
                        # smaller relu2 ops
                        evac(h1[:], l1[:], 1024, relu=True, bias=lb1,
                             engine="act")
                        h1s.append(h1)
                    h1_of[q] = h1s

                def do_L2(q, i):
                    h1 = h1_of[q][i // 2]
                    hoff = 512 * (i % 2)
                    l2 = l2ps.tile([128, 512], F32, tag="l2")
                    for c4 in range(4):
                        nc.tensor.matmul(
                            l2[:, 128 * c4:128 * c4 + 128],
                            h1[:, hoff + 128 * c4:hoff + 128 * c4 + 128],
                            w2[:],
                            start=True, stop=True,
                        )
                        pe_clock[0] += 53.3
                    h2 = h2p.tile([128, 512], F8, tag="h2")
                    evac(h2[:], l2[:], 512, relu=True)
                    h2_of.setdefault(q, []).append(h2)

                def do_seg(q, i):
                    srv, ql = srv_of[q]
                    h2v = h2_of[q][i][:].rearrange("p (i m) -> p i m", i=4)
                    for p2 in range(2):
                        cpid = (q * 4 + i) * 2 + p2
                        w = cpid // ppw
                        first = (cpid % ppw == 0)
                        last = (cpid % ppw == ppw - 1)
                        # aggT_w += h2_pair.T @ S_row_pair (fp8 DoubleRow:
                        # K = 256 rows in one instruction).  All 16 windows
                        # live side by side in one persistent PSUM bank;
                        # start only zeroes the bytes this matmul writes.
                        nc.tensor.matmul(
                            agg_tile[:, W_SEGS * w:W_SEGS * (w + 1)],
                            h2v[:, 2 * p2:2 * p2 + 2, :],
                            srv[:, 16 * ql + 4 * i + 2 * p2:
                                16 * ql + 4 * i + 2 * p2 + 2, :],
                            start=first, stop=last, perf_mode=DR,
                        )
                        pe_clock[0] += 6.7
                        if last and w == N_WINDOWS - 1:
                            evac(aggT[:], agg_tile[:], 512, relu=False,
                                 engine="dve")

                SRQ = 16 * W_SEGS
                group_start = {}
                s = 0
                for g in groups:
                    group_start[s] = g
                    s += g

                for q in range(n_quads + 2):
                    if q < n_quads:
                        if q in group_start:
                            g = group_start[q]
                            q0 = q
                            xt = xtp.tile([16, G * 4096], F8, tag="xt")
                            nc.sync.dma_start(
                                xt[:, :g * 4096],
                                xt4[:, q * 4096:(q + g) * 4096])
                            sr = srp.tile([128, G * SRQ], F8, tag="sr")
                            nc.sync.dma_start(
                                sr[:, :g * SRQ],
                                srowd[:, q * SRQ:(q + g) * SRQ])
                            xtv = xt[:].rearrange("k (q r) -> k q r", r=2048)
                            srv = sr[:].rearrange("p (c j) -> p c j", j=W_SEGS)
                        srv_of[q] = (srv, q - q0)
                        do_L1(q, xtv, q - q0)
                    for i in range(4):
                        if 1 <= q < n_quads + 1:
                            do_L2(q - 1, i)
                        if q >= 2:
                            do_seg(q - 2, i)
                    h1_of.pop(q - 2, None)
                    h2_of.pop(q - 3, None)
                    srv_of.pop(q - 3, None)

            # tail-only constants (deferred DMAs)
            w3 = cload(lw3, [E, E], BF16, "w3")
            g1w = cload(gw1, [E, E], BF16, "g1w")
            g2w = cload(gw2, [E, E], BF16, "g2w")
            g3w = cload(gw3, [E, C], BF16, "g3w")
            idf = cload(identf, [128, 128], F32, "idf")
            gb1 = cload(gb1d, [E, 1], F32, "gb1")
            gb2 = cload(gb2d, [E, 1], F32, "gb2")
            gb3 = cload(gb3d, [C, 1], F32, "gb3")

            # ---------- tail: deferred layer-3 + global MLP ----------
            with ExitStack() as tctx:
                tailps = tctx.enter_context(
                    tc.tile_pool(name="tailps", bufs=2, space="PSUM"))

                ec_flip = 0

                def evac_copy(dst, src):
                    nonlocal ec_flip
                    ec_flip += 1
                    if ec_flip % 2:
                        nc.scalar.copy(dst, src)
                    else:
                        nc.vector.tensor_copy(dst, src)

                # half-split column-pipelined global MLP: the 512 segment
                # columns are independent, so run two 256-column chains and
                # interleave layers for latency
                def emit_half(rhs_tile, w_tile, func, bias, o, hh,
                              out_cols=E):
                    ps = tailps.tile([out_cols, 256], F32, tag="lps")
                    nc.tensor.matmul(ps[:], w_tile[:],
                                     rhs_tile[:, 256 * hh:256 * hh + 256],
                                     start=True, stop=True)
                    dst = o[:, 256 * hh:256 * hh + 256]
                    if func is None:
                        evac_copy(dst, ps[:])
                    elif func == AF.Relu:
                        if bias is not None:
                            nc.vector.tensor_scalar(dst, ps[:], bias[:], 0.0,
                                                    ALU.add, ALU.max)
                        else:
                            nc.vector.tensor_scalar_max(dst, ps[:], 0.0)
                    elif func == AF.Identity and bias is not None:
                        nc.vector.tensor_scalar_add(dst, ps[:], bias[:])
                    else:
                        nc.scalar.activation(
                            dst, ps[:], func,
                            bias=bias[:] if bias is not None else 0.0)

                a3T = tailp.tile([E, 512], BF16, tag="a3T")
                g1T = tailp.tile([E, 512], BF16, tag="g1T")
                g2T = tailp.tile([E, 512], BF16, tag="g2T")
                scT = tailp.tile([C, 512], F32, tag="scT")
                chain = [
                    (aggT, w3, None, None, a3T, E),
                    (a3T, g1w, AF.Relu, gb1, g1T, E),
                    (g1T, g2w, AF.Relu, gb2, g2T, E),
                    (g2T, g3w, AF.Identity, gb3, scT, C),
                ]
                for step in range(5):
                    for li, (rhs, wt, fn, bi, o, ocols) in enumerate(chain):
                        hh = step - li
                        if hh in (0, 1):
                            emit_half(rhs, wt, fn, bi, o, hh, out_cols=ocols)

                # log-softmax, phase-batched so ACT loads the exp table once
                # and the ln table once
                outsb = tailp.tile([128, 4 * C], F32, tag="outsb")
                xs = tailp.tile([128, 4 * C], F32, tag="xs")
                exs = tailp.tile([128, 4 * C], F32, tag="exs")
                negmax = tailp.tile([128, 4], F32, tag="negmax")
                ssum = tailp.tile([128, 4], F32, tag="ssum")
                lse = tailp.tile([128, 4], F32, tag="lse")
                shift = tailp.tile([128, 4], F32, tag="shift")
                mx = tailp.tile([128, 4], F32, tag="mx")
                for t in range(4):
                    sp = tailps.tile([128, C], F32, tag="sp")
                    nc.tensor.transpose(sp[:], scT[:, 128 * t:128 * t + 128],
                                        idf[:C, :C])
                    nc.vector.tensor_copy(xs[:, C * t:C * (t + 1)], sp[:])
                    nc.vector.tensor_reduce(mx[:, t:t + 1],
                                            xs[:, C * t:C * (t + 1)],
                                            mybir.AxisListType.X, ALU.max)
                for t in range(4):
                    nc.vector.tensor_scalar_mul(negmax[:, t:t + 1],
                                                mx[:, t:t + 1], -1.0)
                    nc.scalar.activation(exs[:, C * t:C * (t + 1)],
                                         xs[:, C * t:C * (t + 1)], AF.Exp,
                                         bias=negmax[:, t:t + 1])
                    nc.vector.reduce_sum(ssum[:, t:t + 1],
                                         exs[:, C * t:C * (t + 1)],
                                         axis=mybir.AxisListType.X)
                nc.scalar.activation(lse[:], ssum[:], AF.Ln)
                nc.vector.tensor_tensor(shift[:], negmax[:], lse[:],
                                        op=ALU.subtract)
                for t in range(4):
                    nc.vector.tensor_scalar_add(outsb[:, C * t:C * (t + 1)],
                                                xs[:, C * t:C * (t + 1)],
                                                shift[:, t:t + 1])

                outv = out_ap.rearrange("(t p) c -> p t c", p=128)
                nc.sync.dma_start(
                    outv, outsb[:].rearrange("p (t c) -> p t c", c=C))

    nc.compile()
    return nc, G, R


def _prep_core(x, index_local, core, w_rows, R):
    """Per-core xt4 + srow tensors (fp8)."""
    segs0 = core * SEGS_PER_CORE
    seg_of_row = index_local - segs0

    # destination row: window-contiguous with per-window padding to w_rows
    win_of_row = seg_of_row // W_SEGS
    win_cnt = np.bincount(win_of_row, minlength=N_WINDOWS)
    win_orig_start = np.concatenate(([0], np.cumsum(win_cnt)[:-1]))
    dest = win_of_row * w_rows + (np.arange(len(index_local))
                                  - win_orig_start[win_of_row])
    xpad = np.zeros((R, C), dtype=np.float32)
    xpad[dest] = x

    n_quads = R // 2048
    # x packed for DoubleRow: per quad [16, 2, 2048] (c = i*16 + k),
    # flattened to [16, n_quads*4096]
    xq = xpad.reshape(n_quads, 2048, 2, 16).transpose(0, 3, 2, 1)
    xt4 = xq.transpose(1, 0, 2, 3).reshape(16, n_quads * 4096)

    # per-row one-hot vs window-relative segment id (pad rows get all-zero)
    d = np.full(R, -(10 ** 6), dtype=np.int64)
    d[dest] = seg_of_row - win_of_row * W_SEGS
    srow = (d[:, None] == np.arange(W_SEGS)[None, :])      # [R, 32]
    n_chunks = R // 128
    srow = srow.reshape(n_chunks, 128, W_SEGS).transpose(1, 0, 2)
    srow = srow.reshape(128, n_chunks * W_SEGS)
    return _nf8(xt4), _nf8(srow.astype(np.float32))


def kernel(**inputs) -> np.ndarray:
    x = np.asarray(inputs["x"], dtype=np.float32)
    index = np.asarray(inputs["index"]).astype(np.int64)
    ws = {k: np.asarray(inputs[k], dtype=np.float32)
          for k in ("lW1", "lb1", "lW2", "lb2", "lW3", "lb3",
                    "gW1", "gb1", "gW2", "gb2", "gW3", "gb3")}

    # lb2 enters per-row on the free axis, lb3 would need per-segment counts;
    # both are zero for this model.
    assert not ws["lb2"].any() and not ws["lb3"].any(), \
        "nonzero lb2/lb3 not supported by this kernel"

    if not np.all(index[:-1] <= index[1:]):
        order = np.argsort(index, kind="stable")
        index = index[order]
        x = x[order]

    counts = np.bincount(index, minlength=NUM_ELECTIONS)
    ptr = np.concatenate(([0], np.cumsum(counts)))

    # rows per (core, window), padded to the global max (256-aligned)
    win_rows = counts.reshape(N_CORES * N_WINDOWS, W_SEGS).sum(axis=1)
    w_rows = int(-(-win_rows.max() // 256) * 256)

    nc, G, R = _build_program(w_rows)

    # w1p[k, i*128+e] = W1[i*16+k, e]
    w1p = ws["lW1"].reshape(2, 16, E).transpose(1, 0, 2).reshape(16, 256)

    common = {
        "w1p": _nf8(w1p),
        "lw2": _nf8(ws["lW2"]),
        "lw3": _nb16(ws["lW3"]),
        "gw1": _nb16(ws["gW1"]),
        "gw2": _nb16(ws["gW2"]),
        "gw3": _nb16(ws["gW3"]),
        "identf": np.eye(128, dtype=np.float32),
        "lb1": ws["lb1"].reshape(E, 1).astype(np.float32),
        "gb1": ws["gb1"].reshape(E, 1).astype(np.float32),
        "gb2": ws["gb2"].reshape(E, 1).astype(np.float32),
        "gb3": ws["gb3"].reshape(C, 1).astype(np.float32),
    }

    in_maps = []
    for k in range(N_CORES):
        lo, hi = ptr[k * SEGS_PER_CORE], ptr[(k + 1) * SEGS_PER_CORE]
        xt4, srow = _prep_core(x[lo:hi], index[lo:hi], k, w_rows, R)
        in_maps.append({"xt4": xt4, "srow": srow, **common})

    res = bass_utils.run_bass_kernel_spmd(nc, in_maps, core_ids=list(range(N_CORES)))
    global LAST_RESULTS, LAST_NC, LAST_IN_MAPS
    LAST_RESULTS, LAST_NC, LAST_IN_MAPS = res, nc, in_maps
    out = np.concatenate([res.results[k]["out"] for k in range(N_CORES)], axis=0)
    return out.astype(np.float32)


LAST_RESULTS = None
LAST_NC = None
LAST_IN_MAPS = None


if __name__ == "__main__":
    rng = np.random.default_rng(0)
    idx = np.sort(rng.integers(0, NUM_ELECTIONS, size=N_VOTERS)).astype(np.int64)
    demo = {
        "x": rng.standard_normal((N_VOTERS, C), dtype=np.float32),
        "index": idx,
    }
    for n, sh in (("lW1", (C, E)), ("lW2", (E, E)), ("lW3", (E, E)),
                  ("gW1", (E, E)), ("gW2", (E, E)), ("gW3", (E, C))):
        demo[n] = (rng.standard_normal(sh, dtype=np.float32) * 0.05)
    for n, sh in (("lb1", E), ("lb2", E), ("lb3", E),
                  ("gb1", E), ("gb2", E), ("gb3", C)):
        demo[n] = np.zeros(sh, np.float32)
    out = kernel(**demo)
    print(out.shape, out.dtype, np.isfinite(out).all())
